# revision 1
# baseline (speedup 1.0000x reference)
"""Trainium2 Bass kernel for nn_CausalSelfAttention (quartet-gated, row-normed).

Sharding: head-parallel across 8 cores (2 heads/core, both batches).
Each core computes its head-slice projections, scores, softmax, AV, and a
partial y @ Wo.T over its 128-wide C-slice; host sums the 8 fp32 partials.
"""

import sys

sys.path.insert(0, "/opt/trn_rl_repo")

import numpy as np
import concourse.bass as bass
import concourse.mybir as mybir
import concourse.tile as tile
from concourse.bass_utils import run_bass_kernel_spmd

DT = mybir.dt
AF = mybir.ActivationFunctionType
OP = mybir.AluOpType

B = 2
T = 1024
C = 1024
H = 16
DH = 64
NCORES = 8
H2 = 2  # heads per core
BT = B * T
NB = T // 128  # 8 tq/tk blocks per pair
EPS = 1e-5
SCALE = 1.0 / 8.0  # 1/sqrt(Dh)
EXP_BIAS = -4.0


def _split_multi_waits(nc):
    """This walrus build accepts at most one sync-wait per instruction; hoist
    extras onto preceding wait-only EventSemaphore instructions."""
    n = 0
    for func in nc.m.functions:
        for block in func.blocks:
            new_insts = []
            for inst in block.instructions:
                si = inst.sync_info
                if si is not None and len(si.on_wait) > 1:
                    waits = list(si.on_wait)
                    for w in waits[:-1]:
                        n += 1
                        new_insts.append(
                            mybir.InstEventSemaphore(
                                name=f"waitsplit-{n}",
                                engine=inst.engine,
                                sync_info=mybir.SyncInfo(on_wait=[w], on_update=[]),
                            )
                        )
                    inst.sync_info = mybir.SyncInfo(
                        on_wait=[waits[-1]], on_update=list(si.on_update)
                    )
                new_insts.append(inst)
            block.instructions[:] = new_insts


def _build_program():
    nc = bass.Bass("TRN2", target_bir_lowering=False, debug=False, num_devices=NCORES)

    f16 = DT.float16
    f32 = DT.float32

    xT_d = nc.dram_tensor("xT", [C, BT], f16, kind="ExternalInput").ap()
    w_d = {
        p: nc.dram_tensor(f"w{p}", [C, 128], f16, kind="ExternalInput").ap()
        for p in ("q", "k", "v", "q2", "k2")
    }
    woT_d = nc.dram_tensor("woT", [128, C], f16, kind="ExternalInput").ap()
    id64_d = nc.dram_tensor("id64", [128, 64], f16, kind="ExternalInput").ap()
    trineg_d = nc.dram_tensor("trineg", [128, 128], f16, kind="ExternalInput").ap()
    mqs_d = nc.dram_tensor("mqs", [128, 1], f32, kind="ExternalInput").ap()
    onem_d = nc.dram_tensor("onem", [128, 1], f32, kind="ExternalInput").ap()
    out_d = nc.dram_tensor("out", [BT, C], f32, kind="ExternalOutput").ap()

    from contextlib import ExitStack

    with tile.TileContext(nc) as tc, ExitStack() as es:
        consts = es.enter_context(tc.tile_pool(name="consts", bufs=1))
        projp = es.enter_context(tc.tile_pool(name="projp", bufs=1))
        xpool = es.enter_context(tc.tile_pool(name="xpool", bufs=1))
        ps_proj = tc.alloc_tile_pool(name="ps_proj", bufs=3, space="PSUM")

        # ---- constants ----
        wts = {}
        for p in ("q", "k", "v", "q2", "k2"):
            wts[p] = consts.tile([128, 8, 128], f16, tag=f"w{p}", name=f"w{p}")
            nc.sync.dma_start(
                out=wts[p], in_=w_d[p].rearrange("(kc p) m -> p kc m", p=128)
            )
        mqs = consts.tile([128, 1], f32, tag="mqs", name="mqs")
        nc.sync.dma_start(out=mqs, in_=mqs_d)
        onem = consts.tile([128, 1], f32, tag="onem", name="onem")
        nc.sync.dma_start(out=onem, in_=onem_d)
        ones16 = consts.tile([128, 1], f16, tag="ones16", name="ones16")
        nc.vector.memset(ones16, 1.0)
        expb = consts.tile([128, 1], f32, tag="expb", name="expb")
        nc.vector.memset(expb, EXP_BIAS)

        # ---- phase 1: projections (out: pT [128 (2h x dh), BT] fp16) ----
        xch = {}
        for b in range(B):
            for kc in range(8):
                xch[(b, kc)] = xpool.tile(
                    [128, T], f16, tag=f"x{b}_{kc}", name=f"x{b}_{kc}"
                )
                nc.sync.dma_start(
                    out=xch[(b, kc)],
                    in_=xT_d[kc * 128 : (kc + 1) * 128, b * T : (b + 1) * T],
                )
        id64 = consts.tile([128, 64], f16, tag="id64", name="id64")
        nc.sync.dma_start(out=id64, in_=id64_d)
        trineg = consts.tile([128, 128], f16, tag="trineg", name="trineg")
        nc.sync.dma_start(out=trineg, in_=trineg_d)
        woT = consts.tile([128, C], f16, tag="woT", name="woT")
        nc.sync.dma_start(out=woT, in_=woT_d)
        projT = {}
        for p in ("q", "k", "v", "q2", "k2"):
            projT[p] = projp.tile([128, BT], f16, tag=f"{p}T", name=f"{p}T")
        # scale 1/8 folded into qT and q2T at PSUM->SBUF copy
        for b in range(B):
            for ip, p in enumerate(("k", "k2", "v", "q", "q2")):
                pps = ps_proj.tile([128, T], f32, tag="proj_ps", name="proj_ps")
                for n in range(2):
                    for kc in range(8):
                        nc.tensor.matmul(
                            pps[:, n * 512 : (n + 1) * 512],
                            wts[p][:, kc, :],
                            xch[(b, kc)][:, n * 512 : (n + 1) * 512],
                            start=(kc == 0),
                            stop=(kc == 7),
                        )
                sc = SCALE if p in ("q", "q2") else 1.0
                dst = projT[p][:, b * T : (b + 1) * T]
                if ip % 2 == 0:
                    nc.scalar.activation(dst, pps, AF.Copy, scale=sc)
                else:
                    if sc == 1.0:
                        nc.vector.tensor_copy(dst, pps)
                    else:
                        nc.vector.tensor_scalar_mul(dst, pps, sc)

        ps_proj.release()
        ps_misc = es.enter_context(tc.tile_pool(name="ps_misc", bufs=1, space="PSUM"))
        ps_sc = es.enter_context(tc.tile_pool(name="ps_sc", bufs=5, space="PSUM"))
        natp = es.enter_context(tc.tile_pool(name="natp", bufs=1))
        statp = es.enter_context(tc.tile_pool(name="statp", bufs=1))
        etp = es.enter_context(tc.tile_pool(name="etp", bufs=3))
        workp = es.enter_context(tc.tile_pool(name="workp", bufs=4))
        yp = es.enter_context(tc.tile_pool(name="yp", bufs=1))
        outp = es.enter_context(tc.tile_pool(name="outp", bufs=3))

        pairs = [(b, h) for b in range(B) for h in range(H2)]

        # ---- per pair: stats precompute, scores->E->E^T, AV; Wo per b ----
        k_nat = {}
        vo = {}
        stats = {}
        y_b = {}
        for b in range(B):
            y_b[b] = yp.tile([128, 8, 128], f16, tag=f"y_{b}", name=f"y_{b}")
        def emit_phase5(b):
            # ---- y^T -> Wo partial -> out ----
            yT = yp.tile([128, T], f16, tag=f"yT_{b}", name=f"yT_{b}")
            nc.sync.dma_start_transpose(
                yT.rearrange("p (j f) -> p j f", j=8), y_b[b]
            )
            for m in range(NB):
                o_sb = outp.tile([128, C], f32, tag="o_sb", name="o_sb")
                for n in range(2):
                    wo_ps = ps_misc.tile(
                        [128, 512], f32, tag="small_ps", bufs=2, name="wo_ps"
                    )
                    nc.tensor.matmul(
                        wo_ps,
                        yT[:, m * 128 : (m + 1) * 128],
                        woT[:, n * 512 : (n + 1) * 512],
                    )
                    dst = o_sb[:, n * 512 : (n + 1) * 512]
                    if (m + n) % 2 == 0:
                        nc.scalar.copy(dst, wo_ps)
                    else:
                        nc.vector.tensor_copy(dst, wo_ps)
                nc.sync.dma_start(
                    out=out_d[b * T + m * 128 : b * T + (m + 1) * 128, :],
                    in_=o_sb,
                )



        stats = {}

        def emit_p2(pi):
            b, h = pairs[pi]
            hs = slice(h * 64, h * 64 + 64)
            bs = slice(b * T, (b + 1) * T)
            # transposes: k, k2 -> nat layout [tk-block 128, dh]; v likewise
            for mat in ("k", "k2", "v"):
                tp = ps_misc.tile([128, 8, 64], f16, tag="tpgz", bufs=1, name="tp")
                for j in range(8):
                    nc.tensor.transpose(
                        tp[:, j, :],
                        projT[mat][hs, b * T + j * 128 : b * T + (j + 1) * 128],
                        id64[hs, :],
                    )
                dst = natp.tile([128, 8, 64], f16, tag=f"nat_{mat}_{pi}", name=f"nat_{mat}_{pi}")
                if mat == "k":
                    nc.scalar.copy(dst, tp)
                else:
                    nc.vector.tensor_copy(dst, tp)
                if mat == "v":
                    vo[pi] = dst
                else:
                    k_nat[(pi, mat)] = dst

            st_ps = ps_misc.tile([128, 32], f32, tag="small_ps", bufs=2, name="st_ps")
            col = 0
            for mat, qn in (("k", "q"), ("k2", "q2")):
                # G = sum_tk k k^T  (fp32 PSUM), folded 1/T on copy-out
                g_ps = ps_misc.tile([64, 64], f32, tag="tpgz", bufs=1, name="g_ps")
                kn = k_nat[(pi, mat)]
                for j in range(8):
                    nc.tensor.matmul(
                        g_ps, kn[:, j, :], kn[:, j, :], start=(j == 0), stop=(j == 7)
                    )
                g_s = statp.tile([128, 64], f16, tag=f"g_{pi}_{mat}", name=f"g_{pi}_{mat}")
                nc.scalar.activation(g_s[0:64, :], g_ps, AF.Copy, scale=1.0 / T)
                nc.scalar.activation(g_s[64:128, :], g_ps, AF.Copy, scale=1.0 / T)
                # kbar = (1/T) sum_tk k  [both heads at once would redo work;
                # slice this pair's rows]
                kb_f32 = statp.tile([128, 1], f32, tag=f"kbf_{pi}_{mat}", name=f"kbf_{pi}_{mat}")
                kb = statp.tile([128, 1], f16, tag=f"kb_{pi}_{mat}", name=f"kb_{pi}_{mat}")
                if mat == "k":
                    nc.vector.tensor_reduce(
                        kb_f32[hs, :], projT[mat][hs, bs],
                        axis=mybir.AxisListType.X, op=OP.add,
                    )
                    nc.scalar.activation(kb[hs, :], kb_f32[hs, :], AF.Copy, scale=1.0 / T)
                else:
                    dump = workp.tile([128, T], f16, tag="dump", name="dump")
                    nc.scalar.activation(
                        dump[hs, :], projT[mat][hs, bs], AF.Copy, scale=1.0 / T,
                        accum_out=kb_f32[hs, :],
                    )
                    nc.vector.tensor_copy(kb[hs, :], kb_f32[hs, :])
                # mu_m = qT-block.T @ kbar  -> st_ps[:, col+m]
                for m in range(NB):
                    nc.tensor.matmul(
                        st_ps[:, col + m : col + m + 1],
                        projT[qn][hs, b * T + m * 128 : b * T + (m + 1) * 128],
                        kb[hs, :],
                    )
                # zT = G @ qT ; w = zT * qT ; ex2_m = colsum(w-block)
                ws = workp.tile([128, T], f16, tag="ws", name="ws")
                for n in range(2):
                    z_ps = ps_sc.tile([128, 512], f32, tag="sc_ps", name="z_ps")
                    nc.tensor.matmul(
                        z_ps[hs, :],
                        g_s[hs, :],
                        projT[qn][hs, b * T + n * 512 : b * T + (n + 1) * 512],
                    )
                    nc.vector.tensor_tensor(
                        ws[hs, n * 512 : (n + 1) * 512],
                        z_ps[hs, :],
                        projT[qn][hs, b * T + n * 512 : b * T + (n + 1) * 512],
                        op=OP.mult,
                    )
                for m in range(NB):
                    nc.tensor.matmul(
                        st_ps[:, col + 8 + m : col + 9 + m],
                        ws[hs, m * 128 : (m + 1) * 128],
                        ones16[hs, :],
                    )
                col += 16

            sts = statp.tile([128, 32], f32, tag=f"sts_{pi}", name=f"sts_{pi}")
            nc.scalar.copy(sts, st_ps)
            ab = {}
            for mi, mat in enumerate(("A", "B")):
                mu = sts[:, mi * 16 : mi * 16 + 8]
                ex2 = sts[:, mi * 16 + 8 : mi * 16 + 16]
                musq = statp.tile([128, 8], f32, tag=f"musq_{pi}_{mi}", name=f"musq_{pi}_{mi}")
                nc.vector.tensor_tensor(musq, mu, mu, op=OP.mult)
                varb = statp.tile([128, 8], f32, tag=f"varb_{pi}_{mi}", name=f"varb_{pi}_{mi}")
                nc.vector.tensor_tensor(varb, ex2, musq, op=OP.subtract)
                lnv = statp.tile([128, 8], f32, tag=f"lnv_{pi}_{mi}", name=f"lnv_{pi}_{mi}")
                nc.scalar.activation(lnv, varb, AF.Ln, scale=float(T) / (T - 1))
                rs = statp.tile([128, 8], f32, tag=f"rs_{pi}_{mi}", name=f"rs_{pi}_{mi}")
                nc.scalar.activation(rs, lnv, AF.Exp, scale=-0.5)
                ab[mat] = (mu, rs)
            muA, rsA = ab["A"]
            muB, rsB = ab["B"]
            b1 = statp.tile([128, 8], f32, tag=f"b1_{pi}", name=f"b1_{pi}")
            nc.vector.tensor_scalar(b1, rsB, mqs, None, op0=OP.mult)
            b2 = statp.tile([128, 8], f32, tag=f"b2_{pi}", name=f"b2_{pi}")
            nc.vector.tensor_tensor(b2, muB, b1, op=OP.mult)
            nc.vector.tensor_scalar(b2, b2, -1.0, onem, op0=OP.mult, op1=OP.add)
            b1a = statp.tile([128, 8], f32, tag=f"b1a_{pi}", name=f"b1a_{pi}")
            nc.vector.tensor_tensor(b1a, b1, rsA, op=OP.mult)
            b2a = statp.tile([128, 8], f32, tag=f"b2a_{pi}", name=f"b2a_{pi}")
            nc.vector.tensor_tensor(b2a, b2, rsA, op=OP.mult)
            stats[pi] = dict(mua=muA, b1a=b1a, b2a=b2a)

        def emit_p34(pi):
            b, h = pairs[pi]
            hs = slice(h * 64, h * 64 + 64)
            st = stats[pi]

            e_T = etp.tile([128, 8, T], f16, tag="e_T", name="e_T")
            rowsums = statp.tile([128, 8], f32, tag=f"rsum_{pi}", name=f"rsum_{pi}")
            for m in range(NB):
                valid = (m + 1) * 128
                nch = (valid + 511) // 512
                qsl = projT["q"][hs, b * T + m * 128 : b * T + (m + 1) * 128]
                q2sl = projT["q2"][hs, b * T + m * 128 : b * T + (m + 1) * 128]
                e_t = workp.tile([128, T], f16, tag="e_t", name="e_t")
                for n in range(nch):
                    c0 = n * 512
                    nn = min(512, valid - c0)
                    ksl = projT["k"][hs, b * T + c0 : b * T + c0 + nn]
                    k2sl = projT["k2"][hs, b * T + c0 : b * T + c0 + nn]
                    a_ps = ps_sc.tile([128, 512], f32, tag="sc_ps", name="a_ps")
                    b_ps = ps_sc.tile([128, 512], f32, tag="sc_ps", name="b_ps")
                    nc.tensor.matmul(a_ps[:, :nn], qsl, ksl)
                    nc.tensor.matmul(b_ps[:, :nn], q2sl, k2sl)
                    t_t = workp.tile([128, 512], f16, tag="t_t", name="t_t")
                    if (m + n) % 2 == 0:
                        nc.scalar.activation(
                            t_t[:, :nn],
                            b_ps[:, :nn],
                            AF.Identity,
                            bias=st["b2a"][:, m : m + 1],
                            scale=st["b1a"][:, m : m + 1],
                        )
                    else:
                        nc.vector.tensor_scalar(
                            t_t[:, :nn],
                            b_ps[:, :nn],
                            st["b1a"][:, m : m + 1],
                            st["b2a"][:, m : m + 1],
                            op0=OP.mult,
                            op1=OP.add,
                        )
                    s_t = workp.tile([128, 512], f16, tag="s_t", name="s_t")
                    nc.vector.scalar_tensor_tensor(
                        s_t[:, :nn],
                        a_ps[:, :nn],
                        st["mua"][:, m : m + 1],
                        t_t[:, :nn],
                        op0=OP.subtract,
                        op1=OP.mult,
                    )
                    if c0 <= m * 128 < c0 + nn:
                        off = m * 128 - c0
                        nc.gpsimd.tensor_tensor(
                            s_t[:, off : off + 128],
                            s_t[:, off : off + 128],
                            trineg,
                            op=OP.add,
                        )
                    if n == 0:
                        acc = rowsums[:, m : m + 1]
                    else:
                        rstmp = statp.tile(
                            [128, 1], f32, tag="rstmp", bufs=2, name="rstmp"
                        )
                        acc = rstmp
                    nc.scalar.activation(
                        e_t[:, c0 : c0 + nn],
                        s_t[:, :nn],
                        AF.Exp,
                        bias=expb,
                        accum_out=acc,
                    )
                    if n > 0:
                        nc.vector.tensor_tensor(
                            rowsums[:, m : m + 1],
                            rowsums[:, m : m + 1],
                            rstmp,
                            op=OP.add,
                        )
                nc.sync.dma_start_transpose(
                    e_T[:, 0 : m + 1, m * 128 : (m + 1) * 128], e_t[:, :valid]
                )
            recips = statp.tile([128, 8], f32, tag=f"recip_{pi}", name=f"recip_{pi}")
            nc.vector.reciprocal(recips, rowsums)
            for m in range(NB):
                av_ps = ps_misc.tile([128, 64], f32, tag="small_ps", bufs=2, name="av_ps")
                for kc in range(m + 1):
                    nc.tensor.matmul(
                        av_ps,
                        e_T[:, kc, m * 128 : (m + 1) * 128],
                        vo[pi][:, kc, :],
                        start=(kc == 0),
                        stop=(kc == m),
                    )
                nc.vector.tensor_scalar_mul(
                    y_b[b][:, m, h * 64 : h * 64 + 64], av_ps, recips[:, m : m + 1]
                )


        emit_p2(0)
        emit_p2(1)
        emit_p34(0)
        emit_p2(2)
        emit_p34(1)
        emit_p2(3)
        emit_phase5(0)
        emit_p34(2)
        emit_p34(3)
        emit_phase5(1)

    _split_multi_waits(nc)
    return nc


_NC_CACHE = None
LAST_RESULT = None


def _make_in_maps(inputs):
    x = np.asarray(inputs["x"], np.float32)
    Wq = np.asarray(inputs["Wq"], np.float32)
    Wk = np.asarray(inputs["Wk"], np.float32)
    Wv = np.asarray(inputs["Wv"], np.float32)
    Wq2 = np.asarray(inputs["Wq2"], np.float32)
    Wk2 = np.asarray(inputs["Wk2"], np.float32)
    Wo = np.asarray(inputs["Wo"], np.float32)
    mixture = np.asarray(inputs["mixture"], np.float32)
    quartet_scale = np.asarray(inputs["quartet_scale"], np.float32)

    m = 1.0 / (1.0 + np.exp(-float(mixture[0])))
    mqs = np.full((128, 1), m * float(quartet_scale[0]), np.float32)
    onem = np.full((128, 1), 1.0 - m, np.float32)

    xT = np.ascontiguousarray(x.reshape(BT, C).T).astype(np.float16)
    id64 = np.concatenate([np.eye(64), np.eye(64)], 0).astype(np.float16)
    trineg = ((np.tril(np.ones((128, 128))) - 1.0) * 50.0).astype(np.float16)

    in_maps = []
    for c in range(NCORES):
        cs = slice(c * 128, (c + 1) * 128)
        in_maps.append(
            {
                "xT": xT,
                "wq": np.ascontiguousarray(Wq[cs, :].T).astype(np.float16),
                "wk": np.ascontiguousarray(Wk[cs, :].T).astype(np.float16),
                "wv": np.ascontiguousarray(Wv[cs, :].T).astype(np.float16),
                "wq2": np.ascontiguousarray(Wq2[cs, :].T).astype(np.float16),
                "wk2": np.ascontiguousarray(Wk2[cs, :].T).astype(np.float16),
                "woT": np.ascontiguousarray(Wo[:, cs].T).astype(np.float16),
                "id64": id64,
                "trineg": trineg,
                "mqs": mqs,
                "onem": onem,
            }
        )

    return in_maps


def kernel(**inputs) -> np.ndarray:
    global _NC_CACHE
    in_maps = _make_in_maps(inputs)
    if _NC_CACHE is None:
        _NC_CACHE = _build_program()
    res = run_bass_kernel_spmd(_NC_CACHE, in_maps, core_ids=list(range(NCORES)))
    global LAST_RESULT
    LAST_RESULT = res
    out = np.zeros((BT, C), np.float32)
    for c in range(NCORES):
        out += res.results[c]["out"]
    return out.reshape(B, T, C).astype(np.float32)


if __name__ == "__main__":
    rng = np.random.default_rng(0)
    ins = {
        "x": rng.standard_normal((B, T, C)).astype(np.float32),
        "Wq": rng.standard_normal((C, C)).astype(np.float32) * 0.02,
        "Wk": rng.standard_normal((C, C)).astype(np.float32) * 0.02,
        "Wv": rng.standard_normal((C, C)).astype(np.float32) * 0.02,
        "Wq2": rng.standard_normal((C, C)).astype(np.float32) * 0.02,
        "Wk2": rng.standard_normal((C, C)).astype(np.float32) * 0.02,
        "Wo": rng.standard_normal((C, C)).astype(np.float32) * 0.02,
        "mixture": np.full((1,), -5.0, np.float32),
        "quartet_scale": np.ones((1,), np.float32),
    }
    y = kernel(**ins)
    print("out", y.shape, y.dtype, float(np.abs(y).max()))



# revision 14
# speedup vs baseline: 1.0104x; 1.0104x over previous
"""Trainium2 Bass kernel for nn_CausalSelfAttention (quartet-gated, row-normed).

Sharding: head-parallel across 8 cores (2 heads/core, both batches). Each core
computes its head-slice projections, scores, softmax, AV, and a partial
y @ Wo.T over its 128-wide C-slice; host sums the 8 fp16 partials.

Score pipeline (2 elementwise passes instead of 3):
  center k, k2 per head  ->  a' = q.k_c and b' = q2.k2_c are row-mean-free,
  so row-norm is a pure per-row scale and
    scores = c1 * [(b' + d) o a']   with per-row scalars
    c1 = m*qs*rsB*rsA,  d = (1-m)/(m*qs) * sigmaB.
  One DVE scalar_tensor_tensor builds X = (b'+d) o a'; the c1 scale and -4
  bias ride free on the Act Exp op.  Row variances come from the G-trick
  (G = K_c K_c^T, ex2 = q^T G q) so only causal score blocks are computed.
q2/k2 projections run as fp8e4 DoubleRow matmuls (2x PE): host pre-scales
W2 by 64 to stay in fp8 normal range; the 1/64 folds into copy-out scales.
"""

import sys

sys.path.insert(0, "/opt/trn_rl_repo")

import math

import ml_dtypes
import numpy as np
import concourse.bass as bass
import concourse.mybir as mybir
import concourse.tile as tile
from concourse.bass_utils import run_bass_kernel_spmd

DT = mybir.dt
AF = mybir.ActivationFunctionType
OP = mybir.AluOpType
PM = mybir.MatmulPerfMode

B = 2
T = 1024
C = 1024
H = 16
DH = 64
NCORES = 8
H2 = 2  # heads per core
BT = B * T
NB = T // 128  # 8 tq/tk blocks
SCALE = 1.0 / 8.0  # 1/sqrt(Dh)
W8SCALE = 64.0  # host pre-scale on Wq2/Wk2 before fp8 cast
EXP_BIAS = -4.0
MASKVAL = -30000.0


def _split_multi_waits(nc):
    """This walrus build accepts at most one sync-wait per instruction; hoist
    extras onto preceding wait-only EventSemaphore instructions."""
    n = 0
    for func in nc.m.functions:
        for block in func.blocks:
            new_insts = []
            for inst in block.instructions:
                si = inst.sync_info
                if si is not None and len(si.on_wait) > 1:
                    waits = list(si.on_wait)
                    for w in waits[:-1]:
                        n += 1
                        new_insts.append(
                            mybir.InstEventSemaphore(
                                name=f"waitsplit-{n}",
                                engine=inst.engine,
                                sync_info=mybir.SyncInfo(on_wait=[w], on_update=[]),
                            )
                        )
                    inst.sync_info = mybir.SyncInfo(
                        on_wait=[waits[-1]], on_update=list(si.on_update)
                    )
                new_insts.append(inst)
            block.instructions[:] = new_insts


def _build_program():
    nc = bass.Bass("TRN2", target_bir_lowering=False, debug=False, num_devices=NCORES)

    f8 = DT.float8e4
    f16 = DT.float16
    f32 = DT.float32

    xT_d = nc.dram_tensor("xT", [C, BT], f16, kind="ExternalInput").ap()
    xb8_d = nc.dram_tensor("xb8", [128, 8, BT], f8, kind="ExternalInput").ap()
    w_d = {
        p: nc.dram_tensor(f"w{p}", [C, 128], f16, kind="ExternalInput").ap()
        for p in ("q", "k", "v")
    }
    w8_d = {
        p: nc.dram_tensor(f"w{p}", [C, 128], f8, kind="ExternalInput").ap()
        for p in ("q2", "k2")
    }
    woT_d = nc.dram_tensor("woT", [128, C], f16, kind="ExternalInput").ap()
    trineg_d = nc.dram_tensor("trineg", [128, 128], f16, kind="ExternalInput").ap()
    # [128,1] f32 host consts: exp bias, ln(m*qs), ln((1-m)/(m*qs))
    expb_d = nc.dram_tensor("expb", [128, 1], f32, kind="ExternalInput").ap()
    lnmqs_d = nc.dram_tensor("lnmqs", [128, 1], f32, kind="ExternalInput").ap()
    lnhomq_d = nc.dram_tensor("lnhomq", [128, 1], f32, kind="ExternalInput").ap()
    out_d = nc.dram_tensor("out", [BT, C], f16, kind="ExternalOutput").ap()

    from contextlib import ExitStack

    with tile.TileContext(nc) as tc, ExitStack() as es:
        consts = es.enter_context(tc.tile_pool(name="consts", bufs=1))
        projp = es.enter_context(tc.tile_pool(name="projp", bufs=1))
        xpool = es.enter_context(tc.tile_pool(name="xpool", bufs=1))
        natp = es.enter_context(tc.tile_pool(name="natp", bufs=2))
        statp = es.enter_context(tc.tile_pool(name="statp", bufs=2))
        etp = es.enter_context(tc.tile_pool(name="etp", bufs=2))
        workp = es.enter_context(tc.tile_pool(name="workp", bufs=3))
        yp = es.enter_context(tc.tile_pool(name="yp", bufs=1))
        outp = es.enter_context(tc.tile_pool(name="outp", bufs=2))
        # PSUM bank budget (8 banks of 2KB/part): proj+wo 2, scores 4, stats 1,
        # g/z/av shared 1
        ps_proj = es.enter_context(tc.tile_pool(name="ps_proj", bufs=2, space="PSUM"))
        ps_sc = es.enter_context(tc.tile_pool(name="ps_sc", bufs=4, space="PSUM"))
        ps_misc = es.enter_context(tc.tile_pool(name="ps_misc", bufs=1, space="PSUM"))

        # ---- constants ----
        wts = {}
        for p in ("k", "q", "v"):
            wts[p] = consts.tile([128, 8, 128], f16, tag=f"w{p}", name=f"w{p}")
            nc.sync.dma_start(
                out=wts[p], in_=w_d[p].rearrange("(kc p) m -> p kc m", p=128)
            )
        for p in ("k2", "q2"):
            wts[p] = consts.tile([128, 8, 128], f8, tag=f"w{p}", name=f"w{p}")
            nc.sync.dma_start(
                out=wts[p], in_=w8_d[p].rearrange("(kc p) m -> p kc m", p=128)
            )
        expb = consts.tile([128, 1], f32, tag="expb", name="expb")
        nc.sync.dma_start(out=expb, in_=expb_d)
        lnmqs = consts.tile([128, 1], f32, tag="lnmqs", name="lnmqs")
        nc.sync.dma_start(out=lnmqs, in_=lnmqs_d)
        lnhomq = consts.tile([128, 1], f32, tag="lnhomq", name="lnhomq")
        nc.sync.dma_start(out=lnhomq, in_=lnhomq_d)
        trineg = consts.tile([128, 128], f16, tag="trineg", name="trineg")
        nc.sync.dma_start(out=trineg, in_=trineg_d)
        woT = consts.tile([128, C], f16, tag="woT", name="woT")
        nc.sync.dma_start(out=woT, in_=woT_d)
        ones16 = consts.tile([128, 1], f16, tag="ones16", name="ones16")
        nc.vector.memset(ones16, 1.0)

        # ---- x loads: fp16 chunks per (b, kc) + fp8 DR tile per batch ----
        xch = {}
        xb8 = {}
        for b in range(B):
            for kc in range(8):
                xch[(b, kc)] = xpool.tile(
                    [128, T], f16, tag=f"x{b}_{kc}", name=f"x{b}_{kc}"
                )
                nc.sync.dma_start(
                    out=xch[(b, kc)],
                    in_=xT_d[kc * 128 : (kc + 1) * 128, b * T : (b + 1) * T],
                )
            xb8[b] = xpool.tile([128, 8, T], f8, tag=f"xb8_{b}", name=f"xb8_{b}")
            nc.sync.dma_start(out=xb8[b], in_=xb8_d[:, :, b * T : (b + 1) * T])

        # ---- per-batch state ----
        projT = {}  # (b, p) -> [128, T] fp16 (p in q, v)
        kc_t = {}  # (b, mat) -> centered [128, T] fp16 (mat in k, k2)
        kbsum = {}  # (b, mat) -> [128, 1] f32 accum of copy-out
        y_b = {}
        for b in range(B):
            y_b[b] = yp.tile([128, 8, 128], f16, tag=f"y_{b}", name=f"y_{b}")

        def emit_proj(b):
            # order: k, k2 first (feed the stats chain), then q, v, q2
            for p in ("k", "k2", "q", "v", "q2"):
                if p in ("k", "k2"):
                    dst = projp.tile(
                        [128, T], f16, tag=f"{p}raw_{b}", name=f"{p}raw_{b}"
                    )
                    acc2 = statp.tile(
                        [128, 2], f32, tag=f"kbs_{p}_{b}", name=f"kbs_{p}_{b}"
                    )
                    kbsum[(b, p)] = acc2
                else:
                    dst = projp.tile([128, T], f16, tag=f"{p}_{b}", name=f"{p}_{b}")
                projT[(b, p)] = dst
                for n in range(2):
                    pps = ps_proj.tile([128, 512], f32, tag="proj_ps", name="proj_ps")
                    if p in ("q2", "k2"):
                        for j in range(4):
                            nc.tensor.matmul(
                                pps,
                                wts[p][:, 2 * j : 2 * j + 2, :],
                                xb8[b][:, 2 * j : 2 * j + 2, n * 512 : (n + 1) * 512],
                                start=(j == 0),
                                stop=(j == 3),
                                perf_mode=PM.DoubleRow,
                            )
                    else:
                        for kc in range(8):
                            nc.tensor.matmul(
                                pps,
                                wts[p][:, kc, :],
                                xch[(b, kc)][:, n * 512 : (n + 1) * 512],
                                start=(kc == 0),
                                stop=(kc == 7),
                            )
                    half = dst[:, n * 512 : (n + 1) * 512]
                    if p in ("k", "k2"):
                        sc = 1.0 if p == "k" else 1.0 / W8SCALE
                        nc.scalar.activation(
                            half, pps, AF.Copy, scale=sc,
                            accum_out=kbsum[(b, p)][:, n : n + 1],
                        )
                    elif p == "q":
                        nc.vector.tensor_scalar_mul(half, pps, SCALE)
                    elif p == "v":
                        nc.vector.tensor_copy(half, pps)
                    else:  # q2
                        nc.vector.tensor_scalar_mul(half, pps, SCALE / W8SCALE)

        def emit_center(b):
            for mat in ("k", "k2"):
                kb1 = statp.tile(
                    [128, 1], f32, tag=f"kb1_{mat}_{b}", name=f"kb1_{mat}_{b}"
                )
                nc.vector.tensor_reduce(
                    kb1, kbsum[(b, mat)], axis=mybir.AxisListType.X, op=OP.add
                )
                kbsc = statp.tile(
                    [128, 1], f32, tag=f"kbsc_{mat}_{b}", name=f"kbsc_{mat}_{b}"
                )
                nc.scalar.activation(kbsc, kb1, AF.Copy, scale=1.0 / T)
                cen = projp.tile([128, T], f16, tag=f"{mat}c_{b}", name=f"{mat}c_{b}")
                nc.vector.tensor_scalar(
                    cen, projT[(b, mat)], kbsc, None, op0=OP.subtract
                )
                kc_t[(b, mat)] = cen

        pairs = [(b, h) for b in range(B) for h in range(H2)]
        nat = {}
        stats = {}

        def emit_stats(b):
            """Row-variance stats for BOTH heads of batch b at once
            (z/ws span the full 128 partitions)."""
            # nat transposes (DMA): k_c, k2_c for G; v for AV — full 128 rows
            for mat, src in (
                ("k", kc_t[(b, "k")]),
                ("k2", kc_t[(b, "k2")]),
                ("v", projT[(b, "v")]),
            ):
                dst = natp.tile(
                    [128, 8, 128], f16, tag=f"nat_{mat}", name=f"nat_{mat}_{b}"
                )
                nc.sync.dma_start_transpose(dst, src)
                nat[(b, mat)] = dst

            sts = {}
            for mi, (mat, qn) in enumerate((("k", "q"), ("k2", "q2"))):
                st_ps = ps_misc.tile(
                    [128, 16], f32, tag="st_ps", bufs=1, name="st_ps"
                )
                kn = nat[(b, mat)]
                g_s = statp.tile([128, 64], f16, tag=f"g_{mi}", name=f"g_{b}_{mat}")
                for h in range(H2):
                    hs = slice(h * 64, h * 64 + 64)
                    g_ps = ps_misc.tile(
                        [64, 64], f32, tag="gza_ps", bufs=1, name="g_ps"
                    )
                    for j in range(8):
                        nc.tensor.matmul(
                            g_ps,
                            kn[:, j, hs],
                            kn[:, j, hs],
                            start=(j == 0),
                            stop=(j == 7),
                        )
                    nc.scalar.activation(g_s[hs, :], g_ps, AF.Copy, scale=1.0 / T)
                # z = G q (both heads), ws = z o q, ex2_h = colsums of ws[hs]
                qt = projT[(b, qn)]
                ws = workp.tile([128, T], f16, tag="ws", name="ws")
                for n in range(2):
                    z_ps = ps_misc.tile(
                        [128, 512], f32, tag="gza_ps", bufs=1, name="z_ps"
                    )
                    for h in range(H2):
                        hs = slice(h * 64, h * 64 + 64)
                        nc.tensor.matmul(
                            z_ps[hs, :],
                            g_s[hs, :],
                            qt[hs, n * 512 : (n + 1) * 512],
                        )
                    nc.vector.tensor_tensor(
                        ws[:, n * 512 : (n + 1) * 512],
                        z_ps,
                        qt[:, n * 512 : (n + 1) * 512],
                        op=OP.mult,
                    )
                for h in range(H2):
                    hs = slice(h * 64, h * 64 + 64)
                    for m in range(NB):
                        nc.tensor.matmul(
                            st_ps[:, h * 8 + m : h * 8 + m + 1],
                            ws[hs, m * 128 : (m + 1) * 128],
                            ones16[hs, :],
                        )
                stv = statp.tile([128, 16], f32, tag=f"sts_{mi}", name=f"sts_{b}_{mi}")
                nc.scalar.copy(stv, st_ps)
                sts[mat] = stv  # cols h*8+m = ex2/T for head h, block m

            for h in range(H2):
                pi = pairs.index((b, h))
                cA = sts["k"][:, h * 8 : h * 8 + 8]
                cB = sts["k2"][:, h * 8 : h * 8 + 8]
                lnvA = statp.tile([128, 8], f32, tag="lnvA", name=f"lnvA_{pi}")
                nc.scalar.activation(lnvA, cA, AF.Ln, scale=float(T) / (T - 1))
                lnvB = statp.tile([128, 8], f32, tag="lnvB", name=f"lnvB_{pi}")
                nc.scalar.activation(lnvB, cB, AF.Ln, scale=float(T) / (T - 1))
                lnvAB = statp.tile([128, 8], f32, tag="lnvAB", name=f"lnvAB_{pi}")
                nc.vector.tensor_tensor(lnvAB, lnvA, lnvB, op=OP.add)
                c1 = statp.tile([128, 8], f32, tag=f"c1_{pi}", name=f"c1_{pi}")
                nc.scalar.activation(c1, lnvAB, AF.Exp, scale=-0.5, bias=lnmqs)
                dd = statp.tile([128, 8], f32, tag=f"dd_{pi}", name=f"dd_{pi}")
                nc.scalar.activation(dd, lnvB, AF.Exp, scale=0.5, bias=lnhomq)
                stats[pi] = dict(c1=c1, dd=dd)

        def emit_scores(pi):
            b, h = pairs[pi]
            hs = slice(h * 64, h * 64 + 64)
            st = stats[pi]
            qt = projT[(b, "q")]
            q2t = projT[(b, "q2")]
            kt = kc_t[(b, "k")]
            k2t = kc_t[(b, "k2")]

            e_T = etp.tile([128, 8, T], f16, tag="e_T", name=f"e_T_{pi}")
            rowsums = statp.tile([128, 8], f32, tag=f"rsum_{pi}", name=f"rsum_{pi}")
            for m in range(NB):
                valid = (m + 1) * 128
                nch = (valid + 511) // 512
                X = workp.tile([128, T], f16, tag="X", name="X")
                tb = workp.tile([128, T], f16, tag="tb", name="tb")
                for n in range(nch):
                    c0 = n * 512
                    nn = min(512, valid - c0)
                    a_ps = ps_sc.tile([128, 512], f32, tag="sc_ps", name="a_ps")
                    b_ps = ps_sc.tile([128, 512], f32, tag="sc_ps", name="b_ps")
                    nc.tensor.matmul(
                        b_ps[:, :nn],
                        q2t[hs, m * 128 : (m + 1) * 128],
                        k2t[hs, c0 : c0 + nn],
                    )
                    nc.tensor.matmul(
                        a_ps[:, :nn],
                        qt[hs, m * 128 : (m + 1) * 128],
                        kt[hs, c0 : c0 + nn],
                    )
                    # pass 1: tb = b' + d  (PSUM -> SBUF fp16)
                    if (m + n) % 2 == 0:
                        nc.scalar.activation(
                            tb[:, c0 : c0 + nn],
                            b_ps[:, :nn],
                            AF.Identity,
                            bias=st["dd"][:, m : m + 1],
                        )
                    else:
                        nc.vector.tensor_scalar(
                            tb[:, c0 : c0 + nn],
                            b_ps[:, :nn],
                            st["dd"][:, m : m + 1],
                            None,
                            op0=OP.add,
                        )
                    # pass 2: X = tb o a'  (one PSUM operand)
                    nc.vector.tensor_tensor(
                        X[:, c0 : c0 + nn],
                        a_ps[:, :nn],
                        tb[:, c0 : c0 + nn],
                        op=OP.mult,
                    )
                nc.gpsimd.tensor_tensor(
                    X[:, m * 128 : (m + 1) * 128],
                    X[:, m * 128 : (m + 1) * 128],
                    trineg,
                    op=OP.add,
                )
                e_t = workp.tile([128, T], f16, tag="e_t", name="e_t")
                nc.scalar.activation(
                    e_t[:, :valid],
                    X[:, :valid],
                    AF.Exp,
                    bias=expb,
                    scale=st["c1"][:, m : m + 1],
                    accum_out=rowsums[:, m : m + 1],
                )
                nc.sync.dma_start_transpose(
                    e_T[:, 0 : m + 1, m * 128 : (m + 1) * 128], e_t[:, :valid]
                )
            recips = statp.tile([128, 8], f32, tag=f"recip_{pi}", name=f"recip_{pi}")
            nc.vector.reciprocal(recips, rowsums)
            for m in range(NB):
                av_ps = ps_misc.tile([128, 64], f32, tag="gza_ps", bufs=1, name="av_ps")
                for kcb in range(m + 1):
                    nc.tensor.matmul(
                        av_ps,
                        e_T[:, kcb, m * 128 : (m + 1) * 128],
                        nat[(b, "v")][:, kcb, hs],
                        start=(kcb == 0),
                        stop=(kcb == m),
                    )
                dst = y_b[b][:, m, h * 64 : h * 64 + 64]
                if m % 2 == 0:
                    nc.scalar.activation(
                        dst, av_ps, AF.Copy, scale=recips[:, m : m + 1]
                    )
                else:
                    nc.vector.tensor_scalar_mul(dst, av_ps, recips[:, m : m + 1])

        def emit_wo(b):
            yT = yp.tile([128, T], f16, tag=f"yT_{b}", name=f"yT_{b}")
            nc.sync.dma_start_transpose(
                yT.rearrange("p (j f) -> p j f", j=8), y_b[b]
            )
            for m in range(NB):
                o_sb = outp.tile([128, C], f16, tag="o_sb", name="o_sb")
                for n in range(2):
                    wo_ps = ps_proj.tile(
                        [128, 512], f32, tag="proj_ps", name="wo_ps"
                    )
                    nc.tensor.matmul(
                        wo_ps,
                        yT[:, m * 128 : (m + 1) * 128],
                        woT[:, n * 512 : (n + 1) * 512],
                    )
                    dst = o_sb[:, n * 512 : (n + 1) * 512]
                    if (m + n) % 2 == 0:
                        nc.scalar.copy(dst, wo_ps)
                    else:
                        nc.vector.tensor_copy(dst, wo_ps)
                nc.gpsimd.dma_start(
                    out=out_d[b * T + m * 128 : b * T + (m + 1) * 128, :],
                    in_=o_sb,
                )

        # ---- emission order: interleave batches for cross-phase overlap ----
        emit_proj(0)
        emit_center(0)
        emit_stats(0)
        emit_scores(0)
        emit_proj(1)
        emit_center(1)
        emit_scores(1)
        emit_stats(1)
        emit_wo(0)
        emit_scores(2)
        emit_scores(3)
        emit_wo(1)

    _split_multi_waits(nc)
    return nc


_NC_CACHE = None
LAST_RESULT = None


def _make_in_maps(inputs):
    x = np.asarray(inputs["x"], np.float32)
    Wq = np.asarray(inputs["Wq"], np.float32)
    Wk = np.asarray(inputs["Wk"], np.float32)
    Wv = np.asarray(inputs["Wv"], np.float32)
    Wq2 = np.asarray(inputs["Wq2"], np.float32)
    Wk2 = np.asarray(inputs["Wk2"], np.float32)
    Wo = np.asarray(inputs["Wo"], np.float32)
    mixture = np.asarray(inputs["mixture"], np.float32)
    quartet_scale = np.asarray(inputs["quartet_scale"], np.float32)

    m = 1.0 / (1.0 + np.exp(-float(mixture[0])))
    mqs = m * float(quartet_scale[0])
    expb = np.full((128, 1), EXP_BIAS, np.float32)
    lnmqs = np.full((128, 1), math.log(mqs), np.float32)
    lnhomq = np.full((128, 1), math.log((1.0 - m) / mqs), np.float32)

    xT = np.ascontiguousarray(x.reshape(BT, C).T).astype(np.float16)
    # xb8[p, kc, b*T+t] = x[b, t, kc*128+p]
    xb8 = np.ascontiguousarray(
        xT.reshape(8, 128, BT).transpose(1, 0, 2)
    ).astype(ml_dtypes.float8_e4m3)
    trineg = ((np.tril(np.ones((128, 128))) - 1.0) * -MASKVAL).astype(np.float16)

    in_maps = []
    for c in range(NCORES):
        cs = slice(c * 128, (c + 1) * 128)
        in_maps.append(
            {
                "xT": xT,
                "xb8": xb8,
                "wq": np.ascontiguousarray(Wq[cs, :].T).astype(np.float16),
                "wk": np.ascontiguousarray(Wk[cs, :].T).astype(np.float16),
                "wv": np.ascontiguousarray(Wv[cs, :].T).astype(np.float16),
                "wq2": np.ascontiguousarray(Wq2[cs, :].T * W8SCALE).astype(
                    ml_dtypes.float8_e4m3
                ),
                "wk2": np.ascontiguousarray(Wk2[cs, :].T * W8SCALE).astype(
                    ml_dtypes.float8_e4m3
                ),
                "woT": np.ascontiguousarray(Wo[:, cs].T).astype(np.float16),
                "trineg": trineg,
                "expb": expb,
                "lnmqs": lnmqs,
                "lnhomq": lnhomq,
            }
        )

    return in_maps


def kernel(**inputs) -> np.ndarray:
    global _NC_CACHE
    in_maps = _make_in_maps(inputs)
    if _NC_CACHE is None:
        _NC_CACHE = _build_program()
    res = run_bass_kernel_spmd(_NC_CACHE, in_maps, core_ids=list(range(NCORES)))
    global LAST_RESULT
    LAST_RESULT = res
    out = np.zeros((BT, C), np.float32)
    for c in range(NCORES):
        out += res.results[c]["out"].astype(np.float32)
    return out.reshape(B, T, C)


if __name__ == "__main__":
    rng = np.random.default_rng(0)
    ins = {
        "x": rng.standard_normal((B, T, C)).astype(np.float32),
        "Wq": rng.standard_normal((C, C)).astype(np.float32) * 0.02,
        "Wk": rng.standard_normal((C, C)).astype(np.float32) * 0.02,
        "Wv": rng.standard_normal((C, C)).astype(np.float32) * 0.02,
        "Wq2": rng.standard_normal((C, C)).astype(np.float32) * 0.02,
        "Wk2": rng.standard_normal((C, C)).astype(np.float32) * 0.02,
        "Wo": rng.standard_normal((C, C)).astype(np.float32) * 0.02,
        "mixture": np.full((1,), -5.0, np.float32),
        "quartet_scale": np.ones((1,), np.float32),
    }
    y = kernel(**ins)
    print("out", y.shape, y.dtype, float(np.abs(y).max()))
